# revision 39
# baseline (speedup 1.0000x reference)
"""Trainium2 Bass kernel for batched tiny-projection attention.

Reference computation (per batch b):
    qp = relu(q @ W1.T + b1)            [Nq, 3]
    kp = relu(k @ W2.T + b2)            [Nf, 3]
    scores = (qp @ kp.T) / sqrt(3)      [Nq, Nf]
    attn = softmax(scores, axis=-1)
    out = attn @ v                      [Nq, C]

Shapes: B=4, Nq=2048, Nf=16384, D=3, C=768, fp32.

Algorithm (fast-multipole-style hot/cold split):
  Scores are >= 0, so every exp(score) >= 1 and the softmax denominator is
  >= Nf in absolute units. Hence a polynomial P ~ exp on [0, theta] with
  small ABSOLUTE error gives a tiny relative error on every attention
  weight. P of degree J in the D=3 dot product has only C(J+3,3) monomial
  terms; for J=7 that is R=120 <= 128 partitions, so the entire "cold"
  field (rows whose score stays below theta for every query of the batch)
  collapses into one rank-R pass:
      moments M = Kmono^T @ [v|1]  ([R,769], PE contraction over rows)
      cold contribution = Qmono @ M  (tiny per-chunk matmuls)
  Only "hot" rows (max score > theta; ~7% here) go through the exact
  exp path (fp16 hi/lo score trick -> ACT exp -> bf16 attn matmuls).
  Host computes exact scores (cheap: D=3) to pick hot rows, the exact
  per-batch shift, and the monomial tensors; exp(-shift) is split evenly
  between the Q and K monomial factors and each monomial column is
  power-of-2 balanced so fp16 holds everything in its normal range.

Sharding: 8 cores = (4 batches) x (2 halves of Nq). Softmax is local.
"""

import sys

sys.path.insert(0, "/opt/trn_rl_repo")

import itertools
from math import factorial

import numpy as np

import concourse.bass as bass
import concourse.bacc as bacc
import concourse.tile as tile
from concourse import mybir
from concourse.bass_utils import run_bass_kernel_spmd

F32 = mybir.dt.float32
F16 = mybir.dt.float16
BF16 = mybir.dt.bfloat16

B, NQ_FULL, NF, D, C = 4, 2048, 16384, 3, 768
SCALE = 1.0 / np.sqrt(3.0)
NQ = NQ_FULL // 2          # per-core query rows
CA, CB = 512, C + 1 - 512  # c-chunk split of [v | ones] (769 = 512 + 257)
THETA = 4.0                # hot-score threshold
DEG = 7                    # polynomial degree
ALPHAS = [a for a in itertools.product(range(DEG + 1), repeat=3)
          if sum(a) <= DEG]
RANK = len(ALPHAS)         # 120


def build_nc(nq=NQ, hot_tiles=13, cold_tiles=120, num_devices=8):
    """Single-core SPMD program: hot exact attention + cold rank-RANK pass."""
    assert nq % 512 == 0
    nchunks = nq // 128
    nh = hot_tiles * 128
    caug = C + 1

    nc = bacc.Bacc("TRN2", target_bir_lowering=False, debug=False,
                   num_devices=num_devices)

    assert cold_tiles % 2 == 0
    ccols = caug + RANK        # [v | ones | kmono] packed per cold row
    qT9 = nc.dram_tensor("qT9", [9, nq], F16, kind="ExternalInput")
    kT9 = nc.dram_tensor("kT9", [9, nh], F16, kind="ExternalInput")
    # partition-major: vhot[p, t, :] = hot row t*128+p
    vhot = nc.dram_tensor("vhot", [128, hot_tiles, caug], BF16,
                          kind="ExternalInput")
    wq = nc.dram_tensor("wq", [9, 128], F16, kind="ExternalInput")
    wk = nc.dram_tensor("wk", [9, 128], F16, kind="ExternalInput")
    bq = nc.dram_tensor("bq", [128, 1], F32, kind="ExternalInput")
    bk = nc.dram_tensor("bk", [128, 1], F32, kind="ExternalInput")
    shift = nc.dram_tensor("shift", [128, 1], F32, kind="ExternalInput")
    # partition-major pairs: ccold[p, t, j, :] = cold row t*256+j*128+p
    ccold = nc.dram_tensor("ccold", [128, cold_tiles // 2, 2, ccols], F16,
                           kind="ExternalInput")
    qmono = nc.dram_tensor("qmono", [RANK, nq], F16, kind="ExternalInput")
    out = nc.dram_tensor("out", [nq, C], F32, kind="ExternalOutput")

    with tile.TileContext(nc) as tc, \
         tc.tile_pool(name="const", bufs=1) as const, \
         tc.tile_pool(name="vcp", bufs=12) as vcp, \
         tc.tile_pool(name="vhp", bufs=(hot_tiles + 3) // 4) as vhp, \
         tc.tile_pool(name="expp", bufs=hot_tiles) as expp, \
         tc.tile_pool(name="outp", bufs=2) as outp, \
         tc.tile_pool(name="recp", bufs=2) as recp, \
         tc.tile_pool(name="sc_ps", bufs=2, space="PSUM") as sc_ps, \
         tc.tile_pool(name="oA_ps", bufs=2, space="PSUM") as oA_ps, \
         tc.tile_pool(name="oB_ps", bufs=2, space="PSUM") as oB_ps, \
         tc.tile_pool(name="mom_ps", bufs=1, space="PSUM") as mom_ps:

        # ---- constants / prologue ----
        wq_sb = const.tile([9, 128], F16)
        nc.sync.dma_start(wq_sb[:], wq[:])
        wk_sb = const.tile([9, 128], F16)
        nc.sync.dma_start(wk_sb[:], wk[:])
        bq_sb = const.tile([128, 1], F32)
        nc.sync.dma_start(bq_sb[:], bq[:])
        bk_sb = const.tile([128, 1], F32)
        nc.sync.dma_start(bk_sb[:], bk[:])
        shift_sb = const.tile([128, 1], F32)
        nc.sync.dma_start(shift_sb[:], shift[:])
        qT9_sb = const.tile([9, nq], F16)
        nc.sync.dma_start(qT9_sb[:], qT9[:])
        qmono_sb = const.tile([RANK, nq], F16)
        nc.sync.dma_start(qmono_sb[:], qmono[:])
        kT9_sb = const.tile([9, nh], F16)
        nc.sync.dma_start(kT9_sb[:], kT9[:])

        acc = const.tile([128, nchunks, caug], F32)
        # moments: each core of a batch pair gets HALF the cold tiles in its
        # ccold input (same SPMD program, different data); a pairwise DRAM
        # AllReduce(add) of the [RANK, caug] moment matrix merges the halves.
        npairs = cold_tiles // 2
        mpart_d = nc.dram_tensor("mpart_d", [RANK, caug], F32,
                                 kind="Internal")
        msum_d = nc.dram_tensor("msum_d", [RANK, caug], F32, kind="Internal")
        mprime = const.tile([RANK, caug], F16)
        mpart = const.tile([RANK, caug], F32)
        msum = const.tile([RANK, caug], F32)
        momA = mom_ps.tile([RANK, CA], F32)
        momB = mom_ps.tile([RANK, CB], F32)
        moms_done = []

        def emit_moments(p0, p1):
            """Accumulate cold tile-pairs [p0, p1) into the moments psum."""
            for p in range(p0, p1):
                cc = vcp.tile([128, 2, ccols], F16)
                eng = nc.gpsimd if p % 2 else nc.sync
                eng.dma_start(cc[:], ccold[:, p, :, :])
                for j in range(2):
                    km = cc[:, j, caug:ccols]
                    nc.tensor.matmul(momA[:], km, cc[:, j, 0:CA],
                                     start=(p == 0 and j == 0),
                                     stop=(p == npairs - 1 and j == 1))
                    nc.tensor.matmul(momB[:], km, cc[:, j, CA:caug],
                                     start=(p == 0 and j == 0),
                                     stop=(p == npairs - 1 and j == 1))
            if p1 == npairs and not moms_done:
                moms_done.append(True)
                nc.vector.tensor_copy(mpart[:, 0:CA], momA[:])
                nc.vector.tensor_copy(mpart[:, CA:caug], momB[:])
                nc.sync.dma_start(mpart_d[:], mpart[:])
                nc.gpsimd.collective_compute(
                    "AllReduce", mybir.AluOpType.add,
                    [[0, 1], [2, 3], [4, 5], [6, 7]],
                    [mpart_d[:]], [msum_d[:]])
                nc.sync.dma_start(msum[:], msum_d[:])
                nc.vector.tensor_copy(mprime[:], msum[:])

        def proj_and_split(tag, w_sb, b_sb, rhs_sb, n, lo_ranges):
            """Project rhs [9, n] -> relu'd p32 [128, n] (row blocks at
            {0,32,64,96}), then fp16 split: hi copies + lo residuals."""
            p32 = const.tile([128, n], F32, name=f"{tag}_p32")
            for h0 in range(0, n, 512):
                w = min(512, n - h0)
                pj = sc_ps.tile([128, 512], F32, name="spsum")
                nc.tensor.matmul(pj[:, 0:w], w_sb[:], rhs_sb[:, h0:h0 + w],
                                 start=True, stop=True)
                nc.scalar.activation(p32[:, h0:h0 + w], pj[:, 0:w],
                                     mybir.ActivationFunctionType.Relu,
                                     bias=b_sb[:], scale=1.0)
            hsc = const.tile([128, n], F16, name=f"{tag}_hsc")
            sp = const.tile([128, n], F16, name=f"{tag}_sp")
            nc.vector.tensor_copy(sp[:], p32[:])
            for p0, p1 in lo_ranges:
                nc.scalar.copy(hsc[p0:p1, :], p32[p0:p1, :])
                nc.vector.tensor_sub(sp[p0:p1, :], p32[p0:p1, :],
                                     hsc[p0:p1, :])
            return sp

        # a batch of cold pairs up front: PE work that only waits on DMA
        emit_moments(0, min(10, npairs))

        # projections (q: blocks [hi, lo, hi, lo]; k: blocks [hi, hi, lo, lo])
        qsplit = proj_and_split("q", wq_sb, bq_sb, qT9_sb, nq,
                                lo_ranges=((32, 64), (96, 128)))
        ksplit = proj_and_split("k", wk_sb, bk_sb, kT9_sb, nh,
                                lo_ranges=((64, 128),))

        emit_moments(min(10, npairs), min(14, npairs))

        # hot v tiles (grouped DMAs on the scalar queue) + scores/exp
        vts = []
        for g0 in range(0, hot_tiles, 4):
            gw = min(4, hot_tiles - g0)
            vg = vhp.tile([128, 4, caug], BF16, name="vg")
            nc.scalar.dma_start(vg[:, 0:gw, :], vhot[:, g0:g0 + gw, :])
            for i in range(gw):
                vts.append(vg[:, i, :])

        es = []
        for t in range(hot_tiles):
            et = expp.tile([128, nq], BF16)
            for h in range(nq // 512):
                spsum = sc_ps.tile([128, 512], F32)
                nc.tensor.matmul(spsum[:], ksplit[:, t * 128:(t + 1) * 128],
                                 qsplit[:, h * 512:(h + 1) * 512],
                                 start=True, stop=True)
                nc.scalar.activation(et[:, h * 512:(h + 1) * 512], spsum[:],
                                     mybir.ActivationFunctionType.Exp,
                                     bias=shift_sb[:], scale=float(SCALE))
            es.append(et)

        # hot attention per chunk, moments interleaved to keep DMA flowing;
        # finish moments by chunk 6 so the eval tail overlaps the last chunks
        mom_done = min(14, npairs)
        per = max(1, (npairs - mom_done + 5) // 6)
        for ci in range(nchunks):
            pA = oA_ps.tile([128, CA], F32)
            pB = oB_ps.tile([128, CB], F32)
            for i in range(hot_tiles):
                e = es[i][:, ci * 128:(ci + 1) * 128]
                nc.tensor.matmul(pA[:], e, vts[i][:, 0:CA],
                                 start=(i == 0), stop=(i == hot_tiles - 1))
                nc.tensor.matmul(pB[:], e, vts[i][:, CA:caug],
                                 start=(i == 0), stop=(i == hot_tiles - 1))
            nc.vector.tensor_copy(acc[:, ci, 0:CA], pA[:])
            nc.vector.tensor_copy(acc[:, ci, CA:caug], pB[:])
            m1 = min(mom_done + per, npairs)
            emit_moments(mom_done, m1)
            mom_done = m1
        emit_moments(mom_done, npairs)

        # cold evaluation per chunk: acc += Qmono_chunk^T @ M
        for ci in range(nchunks):
            eA = oA_ps.tile([128, CA], F32, name="pA")
            eB = oB_ps.tile([128, CB], F32, name="pB")
            qm = qmono_sb[:, ci * 128:(ci + 1) * 128]
            nc.tensor.matmul(eA[:], qm, mprime[:, 0:CA],
                             start=True, stop=True)
            nc.tensor.matmul(eB[:], qm, mprime[:, CA:caug],
                             start=True, stop=True)
            nc.vector.tensor_add(acc[:, ci, 0:CA], acc[:, ci, 0:CA], eA[:])
            nc.vector.tensor_add(acc[:, ci, CA:caug], acc[:, ci, CA:caug],
                                 eB[:])
            # finale fused in: normalize and store this chunk
            rec = recp.tile([128, 1], F32)
            nc.vector.reciprocal(rec[:], acc[:, ci, C:caug])
            ot = outp.tile([128, C], F32)
            nc.vector.tensor_scalar_mul(ot[:], acc[:, ci, 0:C], rec[:])
            nc.sync.dma_start(out[ci * 128:(ci + 1) * 128, :], ot[:])

    nc.finalize()
    return nc


def _split16(x):
    hi = x.astype(np.float16)
    lo = (x - hi.astype(np.float32)).astype(np.float16)
    return hi, lo


def _wlhs(W):
    """lhsT [9, 128] for the projection matmul: K rows = [Whi, Whi, Wlo]
    (pairing rhs rows [xhi, xlo, xhi]); output cols 32c+e = projected
    row e replicated on the 4 partition blocks, zeros elsewhere."""
    Whi, Wlo = _split16(W.astype(np.float32))
    m = np.zeros((9, 128), np.float16)
    for e in range(3):
        for d in range(3):
            for cblk in range(4):
                m[0 + d, 32 * cblk + e] = Whi[e, d]
                m[3 + d, 32 * cblk + e] = Whi[e, d]
                m[6 + d, 32 * cblk + e] = Wlo[e, d]
    return m


def _brep(b):
    """bias [128, 1]: b[e] at partitions 32c+e, zero elsewhere."""
    m = np.zeros((128, 1), np.float32)
    for e in range(3):
        for cblk in range(4):
            m[32 * cblk + e, 0] = b[e]
    return m


def _t9(x2d):
    """[N, 3] -> [9, N] fp16 rows [hi, lo, hi]."""
    xT = np.ascontiguousarray(x2d.T.astype(np.float32))
    hi, lo = _split16(xT)
    return np.concatenate([hi, lo, hi], axis=0)


def _cheb_coefs():
    cheb = np.polynomial.chebyshev.Chebyshev.interpolate(
        np.exp, DEG, domain=[0, THETA])
    return cheb.convert(kind=np.polynomial.Polynomial).coef


def _host_prep(q, k, v, W1, b1, W2, b2):
    """Exact host scores -> hot/cold split + monomial tensors."""
    import ml_dtypes
    wq_l, wk_l = _wlhs(W1), _wlhs(W2)
    bq_r, bk_r = _brep(b1), _brep(b2)
    pcoef = _cheb_coefs()

    per_batch = []
    for b in range(B):
        qp = np.maximum(q[b].astype(np.float32) @ W1.T.astype(np.float32)
                        + b1.astype(np.float32), 0.0)
        kp = np.maximum(k[b].astype(np.float32) @ W2.T.astype(np.float32)
                        + b2.astype(np.float32), 0.0)
        s = (qp @ kp.T) * np.float32(SCALE)
        smax = float(s.max())
        hot = s.max(axis=0) > THETA
        per_batch.append((qp, kp, smax, hot))

    hot_tiles = max(-(-int(h.sum()) // 128) for _, _, _, h in per_batch)
    hot_tiles = max(hot_tiles, 1)
    # each core of a batch pair gets half the cold rows
    cold_tiles = max(-(-(int((~h).sum()) + 1) // 2 // 128)
                     for _, _, _, h in per_batch)
    cold_tiles += cold_tiles % 2

    ccols = C + 1 + RANK
    batch_maps = []
    for b in range(B):
        qp, kp, smax, hot = per_batch[b]
        nhot, nh = int(hot.sum()), hot_tiles * 128
        kh = np.zeros((nh, D), np.float32)
        kh[:nhot] = k[b][hot]
        if nhot < nh:  # pad: duplicate k row, vhot stays 0 -> contributes 0
            kh[nhot:] = k[b][0]
        vh = np.zeros((nh, C + 1), np.float32)
        vh[:nhot, :C] = v[b][hot]
        vh[:nhot, C] = 1.0
        vh = np.ascontiguousarray(
            vh.reshape(hot_tiles, 128, C + 1).transpose(1, 0, 2))

        ncold = int((~hot).sum())
        kpc = kp[~hot]
        A = np.exp(-smax / 2.0)
        ccfull = np.zeros((ncold, ccols), np.float32)
        ccfull[:, :C] = v[b][~hot]
        ccfull[:, C] = 1.0
        Qm = np.empty((NQ_FULL, RANK), np.float32)
        for i, a in enumerate(ALPHAS):
            j = a[0] + a[1] + a[2]
            cj = (pcoef[j] * SCALE ** j * factorial(j)
                  / (factorial(a[0]) * factorial(a[1]) * factorial(a[2])))
            kcol = cj * (kpc[:, 0] ** a[0] * kpc[:, 1] ** a[1]
                         * kpc[:, 2] ** a[2]) * A
            qcol = (qp[:, 0] ** a[0] * qp[:, 1] ** a[1]
                    * qp[:, 2] ** a[2]) * A
            km_ = np.abs(kcol).max() + 1e-300
            qm_ = np.abs(qcol).max() + 1e-300
            t = 2.0 ** np.round(0.5 * np.log2(qm_ / km_))
            ccfull[:, C + 1 + i] = kcol * t
            Qm[:, i] = qcol / t

        ncp = cold_tiles * 128
        cch = []
        for h in range(2):
            part = ccfull[h::2]
            cc = np.zeros((ncp, ccols), np.float32)
            cc[:part.shape[0]] = part
            cc = np.ascontiguousarray(
                cc.reshape(cold_tiles // 2, 2, 128, ccols)
                .transpose(2, 0, 1, 3))
            cch.append(cc.astype(np.float16))

        batch_maps.append({
            "kT9": _t9(kh),
            "vhot": vh.astype(ml_dtypes.bfloat16),
            "shift": np.full((128, 1), -smax, np.float32),
            "ccold": cch,
            "Qm": Qm,
        })

    in_maps = []
    for core in range(8):
        b, h = core // 2, core % 2
        bm = batch_maps[b]
        qs = q[b, h * NQ:(h + 1) * NQ, :]
        qmono = np.ascontiguousarray(
            bm["Qm"][h * NQ:(h + 1) * NQ, :].T).astype(np.float16)
        in_maps.append({
            "qT9": _t9(qs), "qmono": qmono,
            "wq": wq_l, "wk": wk_l, "bq": bq_r, "bk": bk_r,
            "kT9": bm["kT9"], "vhot": bm["vhot"], "shift": bm["shift"],
            "ccold": bm["ccold"][h],
        })
    return in_maps, hot_tiles, cold_tiles


_NC_CACHE = {}


def kernel(q, k, v, W1, b1, W2, b2, _trace=False):
    q, k, v = np.asarray(q), np.asarray(k), np.asarray(v)
    W1, b1 = np.asarray(W1), np.asarray(b1)
    W2, b2 = np.asarray(W2), np.asarray(b2)

    in_maps, hot_tiles, cold_tiles = _host_prep(q, k, v, W1, b1, W2, b2)
    key = (hot_tiles, cold_tiles)
    if key not in _NC_CACHE:
        _NC_CACHE[key] = build_nc(hot_tiles=hot_tiles, cold_tiles=cold_tiles)
    nc = _NC_CACHE[key]

    res = run_bass_kernel_spmd(nc, in_maps, list(range(8)), trace=_trace)

    out = np.empty((B, NQ_FULL, C), np.float32)
    for core in range(8):
        b, h = core // 2, core % 2
        out[b, h * NQ:(h + 1) * NQ, :] = res.results[core]["out"]
    if _trace:
        return out, res
    return out


# revision 41
# speedup vs baseline: 1.2892x; 1.2892x over previous
"""Trainium2 Bass kernel for batched tiny-projection attention.

Reference computation (per batch b):
    qp = relu(q @ W1.T + b1)            [Nq, 3]
    kp = relu(k @ W2.T + b2)            [Nf, 3]
    scores = (qp @ kp.T) / sqrt(3)      [Nq, Nf]
    attn = softmax(scores, axis=-1)
    out = attn @ v                      [Nq, C]

Shapes: B=4, Nq=2048, Nf=16384, D=3, C=768, fp32.

Algorithm (fast-multipole-style hot/cold split):
  Scores are >= 0, so every exp(score) >= 1 and the softmax denominator is
  >= Nf in absolute units. Hence a polynomial P ~ exp on [0, theta] with
  small ABSOLUTE error gives a tiny relative error on every attention
  weight. P of degree J in the D=3 dot product has only C(J+3,3) monomial
  terms; for J=7 that is R=120 <= 128 partitions, so the entire "cold"
  field (rows whose score stays below theta for every query of the batch)
  collapses into one rank-R pass:
      moments M = Kmono^T @ [v|1]  ([R,769], PE contraction over rows)
      cold contribution = Qmono @ M  (tiny per-chunk matmuls)
  Only "hot" rows (max score > theta; ~7% here) go through the exact
  exp path (fp16 hi/lo score trick -> ACT exp -> bf16 attn matmuls).
  Host computes exact scores (cheap: D=3) to pick hot rows, the exact
  per-batch shift, and the monomial tensors; exp(-shift) is split evenly
  between the Q and K monomial factors and each monomial column is
  power-of-2 balanced so fp16 holds everything in its normal range.

Sharding: 8 cores = (4 batches) x (2 halves of Nq). Softmax is local.
"""

import sys

sys.path.insert(0, "/opt/trn_rl_repo")

import itertools
from math import factorial

import numpy as np

import concourse.bass as bass
import concourse.bacc as bacc
import concourse.tile as tile
from concourse import mybir
from concourse.bass_utils import run_bass_kernel_spmd

F32 = mybir.dt.float32
F16 = mybir.dt.float16
BF16 = mybir.dt.bfloat16

B, NQ_FULL, NF, D, C = 4, 2048, 16384, 3, 768
SCALE = 1.0 / np.sqrt(3.0)
NQ = NQ_FULL // 2          # per-core query rows
CA, CB = 512, C + 1 - 512  # c-chunk split of [v | ones] (769 = 512 + 257)
THETA = 4.0                # hot-score threshold
DEG = 7                    # polynomial degree
ALPHAS = [a for a in itertools.product(range(DEG + 1), repeat=3)
          if sum(a) <= DEG]
RANK = len(ALPHAS)         # 120


def build_nc(nq=NQ, hot_tiles=13, cold_tiles=120, num_devices=8):
    """Single-core SPMD program: hot exact attention + cold rank-RANK pass."""
    assert nq % 512 == 0
    nchunks = nq // 128
    nh = hot_tiles * 128
    caug = C + 1

    nc = bacc.Bacc("TRN2", target_bir_lowering=False, debug=False,
                   num_devices=num_devices)

    assert cold_tiles % 2 == 0
    ccols = caug + RANK        # [v | ones | kmono] packed per cold row
    qT9 = nc.dram_tensor("qT9", [9, nq], F16, kind="ExternalInput")
    kT9 = nc.dram_tensor("kT9", [9, nh], F16, kind="ExternalInput")
    # partition-major: vhot[p, t, :] = hot row t*128+p
    vhot = nc.dram_tensor("vhot", [128, hot_tiles, caug], BF16,
                          kind="ExternalInput")
    wq = nc.dram_tensor("wq", [9, 128], F16, kind="ExternalInput")
    wk = nc.dram_tensor("wk", [9, 128], F16, kind="ExternalInput")
    bq = nc.dram_tensor("bq", [128, 1], F32, kind="ExternalInput")
    bk = nc.dram_tensor("bk", [128, 1], F32, kind="ExternalInput")
    shift = nc.dram_tensor("shift", [128, 1], F32, kind="ExternalInput")
    # partition-major pairs: ccold[p, t, j, :] = cold row t*256+j*128+p
    ccold = nc.dram_tensor("ccold", [128, cold_tiles // 2, 2, ccols], F16,
                           kind="ExternalInput")
    qmono = nc.dram_tensor("qmono", [RANK, nq], F16, kind="ExternalInput")
    out = nc.dram_tensor("out", [nq, C], F32, kind="ExternalOutput")

    with tile.TileContext(nc) as tc, \
         tc.tile_pool(name="const", bufs=1) as const, \
         tc.tile_pool(name="vcp", bufs=12) as vcp, \
         tc.tile_pool(name="vhp", bufs=(hot_tiles + 3) // 4) as vhp, \
         tc.tile_pool(name="expp", bufs=hot_tiles) as expp, \
         tc.tile_pool(name="outp", bufs=2) as outp, \
         tc.tile_pool(name="recp", bufs=2) as recp, \
         tc.tile_pool(name="sc_ps", bufs=2, space="PSUM") as sc_ps, \
         tc.tile_pool(name="oA_ps", bufs=2, space="PSUM") as oA_ps, \
         tc.tile_pool(name="oB_ps", bufs=2, space="PSUM") as oB_ps, \
         tc.tile_pool(name="mom_ps", bufs=1, space="PSUM") as mom_ps:

        # ---- constants / prologue ----
        wq_sb = const.tile([9, 128], F16)
        nc.sync.dma_start(wq_sb[:], wq[:])
        wk_sb = const.tile([9, 128], F16)
        nc.sync.dma_start(wk_sb[:], wk[:])
        bq_sb = const.tile([128, 1], F32)
        nc.sync.dma_start(bq_sb[:], bq[:])
        bk_sb = const.tile([128, 1], F32)
        nc.sync.dma_start(bk_sb[:], bk[:])
        shift_sb = const.tile([128, 1], F32)
        nc.sync.dma_start(shift_sb[:], shift[:])
        qT9_sb = const.tile([9, nq], F16)
        nc.sync.dma_start(qT9_sb[:], qT9[:])
        qmono_sb = const.tile([RANK, nq], F16)
        nc.sync.dma_start(qmono_sb[:], qmono[:])
        kT9_sb = const.tile([9, nh], F16)
        nc.sync.dma_start(kT9_sb[:], kT9[:])

        acc = const.tile([128, nchunks, caug], F32)
        # moments: each core of a batch pair gets HALF the cold tiles in its
        # ccold input (same SPMD program, different data); a pairwise DRAM
        # AllReduce(add) of the [RANK, caug] moment matrix merges the halves.
        npairs = cold_tiles // 2
        mpart_d = nc.dram_tensor("mpart_d", [RANK, caug], F16,
                                 kind="Internal")
        msum_d = nc.dram_tensor("msum_d", [RANK, caug], F16, kind="Internal")
        mprime = const.tile([RANK, caug], F16)
        mpart = const.tile([RANK, caug], F16)
        momA = mom_ps.tile([RANK, CA], F32)
        momB = mom_ps.tile([RANK, CB], F32)
        moms_done = []

        def emit_moments(p0, p1):
            """Accumulate cold tile-pairs [p0, p1) into the moments psum."""
            for p in range(p0, p1):
                cc = vcp.tile([128, 2, ccols], F16)
                eng = nc.gpsimd if p % 2 else nc.sync
                eng.dma_start(cc[:], ccold[:, p, :, :])
                for j in range(2):
                    km = cc[:, j, caug:ccols]
                    nc.tensor.matmul(momA[:], km, cc[:, j, 0:CA],
                                     start=(p == 0 and j == 0),
                                     stop=(p == npairs - 1 and j == 1))
                    nc.tensor.matmul(momB[:], km, cc[:, j, CA:caug],
                                     start=(p == 0 and j == 0),
                                     stop=(p == npairs - 1 and j == 1))
            if p1 == npairs and not moms_done:
                moms_done.append(True)
                nc.vector.tensor_copy(mpart[:, 0:CA], momA[:])
                nc.vector.tensor_copy(mpart[:, CA:caug], momB[:])
                nc.sync.dma_start(mpart_d[:], mpart[:])
                nc.gpsimd.collective_compute(
                    "AllReduce", mybir.AluOpType.add,
                    [[0, 1], [2, 3], [4, 5], [6, 7]],
                    [mpart_d[:]], [msum_d[:]])
                nc.sync.dma_start(mprime[:], msum_d[:])

        def proj_and_split(tag, w_sb, b_sb, rhs_sb, n, lo_ranges):
            """Project rhs [9, n] -> relu'd p32 [128, n] (row blocks at
            {0,32,64,96}), then fp16 split: hi copies + lo residuals."""
            p32 = const.tile([128, n], F32, name=f"{tag}_p32")
            for h0 in range(0, n, 512):
                w = min(512, n - h0)
                pj = sc_ps.tile([128, 512], F32, name="spsum")
                nc.tensor.matmul(pj[:, 0:w], w_sb[:], rhs_sb[:, h0:h0 + w],
                                 start=True, stop=True)
                nc.scalar.activation(p32[:, h0:h0 + w], pj[:, 0:w],
                                     mybir.ActivationFunctionType.Relu,
                                     bias=b_sb[:], scale=1.0)
            hsc = const.tile([128, n], F16, name=f"{tag}_hsc")
            sp = const.tile([128, n], F16, name=f"{tag}_sp")
            nc.vector.tensor_copy(sp[:], p32[:])
            for p0, p1 in lo_ranges:
                nc.scalar.copy(hsc[p0:p1, :], p32[p0:p1, :])
                nc.vector.tensor_sub(sp[p0:p1, :], p32[p0:p1, :],
                                     hsc[p0:p1, :])
            return sp

        # a batch of cold pairs up front: PE work that only waits on DMA
        emit_moments(0, min(10, npairs))

        # projections (q: blocks [hi, lo, hi, lo]; k: blocks [hi, hi, lo, lo])
        qsplit = proj_and_split("q", wq_sb, bq_sb, qT9_sb, nq,
                                lo_ranges=((32, 64), (96, 128)))
        ksplit = proj_and_split("k", wk_sb, bk_sb, kT9_sb, nh,
                                lo_ranges=((64, 128),))

        # hot v tiles (grouped DMAs on the scalar queue) + scores/exp,
        # moment pairs interleaved; ALL moments finish before hot attention
        # so the AllReduce latency is hidden behind the attention chunks
        vts = []
        for g0 in range(0, hot_tiles, 4):
            gw = min(4, hot_tiles - g0)
            vg = vhp.tile([128, 4, caug], BF16, name="vg")
            nc.scalar.dma_start(vg[:, 0:gw, :], vhot[:, g0:g0 + gw, :])
            for i in range(gw):
                vts.append(vg[:, i, :])

        es = []
        mom_done = min(10, npairs)
        for t in range(hot_tiles):
            et = expp.tile([128, nq], BF16)
            for h in range(nq // 512):
                spsum = sc_ps.tile([128, 512], F32)
                nc.tensor.matmul(spsum[:], ksplit[:, t * 128:(t + 1) * 128],
                                 qsplit[:, h * 512:(h + 1) * 512],
                                 start=True, stop=True)
                nc.scalar.activation(et[:, h * 512:(h + 1) * 512], spsum[:],
                                     mybir.ActivationFunctionType.Exp,
                                     bias=shift_sb[:], scale=float(SCALE))
            es.append(et)
            m1 = min(mom_done + 2, npairs)
            emit_moments(mom_done, m1)
            mom_done = m1
        emit_moments(mom_done, npairs)

        # hot attention per chunk
        for ci in range(nchunks):
            pA = oA_ps.tile([128, CA], F32)
            pB = oB_ps.tile([128, CB], F32)
            for i in range(hot_tiles):
                e = es[i][:, ci * 128:(ci + 1) * 128]
                nc.tensor.matmul(pA[:], e, vts[i][:, 0:CA],
                                 start=(i == 0), stop=(i == hot_tiles - 1))
                nc.tensor.matmul(pB[:], e, vts[i][:, CA:caug],
                                 start=(i == 0), stop=(i == hot_tiles - 1))
            nc.vector.tensor_copy(acc[:, ci, 0:CA], pA[:])
            nc.vector.tensor_copy(acc[:, ci, CA:caug], pB[:])

        # cold evaluation per chunk: acc += Qmono_chunk^T @ M
        for ci in range(nchunks):
            eA = oA_ps.tile([128, CA], F32, name="pA")
            eB = oB_ps.tile([128, CB], F32, name="pB")
            qm = qmono_sb[:, ci * 128:(ci + 1) * 128]
            nc.tensor.matmul(eA[:], qm, mprime[:, 0:CA],
                             start=True, stop=True)
            nc.tensor.matmul(eB[:], qm, mprime[:, CA:caug],
                             start=True, stop=True)
            nc.vector.tensor_add(acc[:, ci, 0:CA], acc[:, ci, 0:CA], eA[:])
            nc.vector.tensor_add(acc[:, ci, CA:caug], acc[:, ci, CA:caug],
                                 eB[:])
            # finale fused in: normalize and store this chunk
            rec = recp.tile([128, 1], F32)
            nc.vector.reciprocal(rec[:], acc[:, ci, C:caug])
            ot = outp.tile([128, C], F32)
            nc.vector.tensor_scalar_mul(ot[:], acc[:, ci, 0:C], rec[:])
            nc.sync.dma_start(out[ci * 128:(ci + 1) * 128, :], ot[:])

    nc.finalize()
    return nc


def _split16(x):
    hi = x.astype(np.float16)
    lo = (x - hi.astype(np.float32)).astype(np.float16)
    return hi, lo


def _wlhs(W):
    """lhsT [9, 128] for the projection matmul: K rows = [Whi, Whi, Wlo]
    (pairing rhs rows [xhi, xlo, xhi]); output cols 32c+e = projected
    row e replicated on the 4 partition blocks, zeros elsewhere."""
    Whi, Wlo = _split16(W.astype(np.float32))
    m = np.zeros((9, 128), np.float16)
    for e in range(3):
        for d in range(3):
            for cblk in range(4):
                m[0 + d, 32 * cblk + e] = Whi[e, d]
                m[3 + d, 32 * cblk + e] = Whi[e, d]
                m[6 + d, 32 * cblk + e] = Wlo[e, d]
    return m


def _brep(b):
    """bias [128, 1]: b[e] at partitions 32c+e, zero elsewhere."""
    m = np.zeros((128, 1), np.float32)
    for e in range(3):
        for cblk in range(4):
            m[32 * cblk + e, 0] = b[e]
    return m


def _t9(x2d):
    """[N, 3] -> [9, N] fp16 rows [hi, lo, hi]."""
    xT = np.ascontiguousarray(x2d.T.astype(np.float32))
    hi, lo = _split16(xT)
    return np.concatenate([hi, lo, hi], axis=0)


def _cheb_coefs():
    cheb = np.polynomial.chebyshev.Chebyshev.interpolate(
        np.exp, DEG, domain=[0, THETA])
    return cheb.convert(kind=np.polynomial.Polynomial).coef


def _host_prep(q, k, v, W1, b1, W2, b2):
    """Exact host scores -> hot/cold split + monomial tensors."""
    import ml_dtypes
    wq_l, wk_l = _wlhs(W1), _wlhs(W2)
    bq_r, bk_r = _brep(b1), _brep(b2)
    pcoef = _cheb_coefs()

    per_batch = []
    for b in range(B):
        qp = np.maximum(q[b].astype(np.float32) @ W1.T.astype(np.float32)
                        + b1.astype(np.float32), 0.0)
        kp = np.maximum(k[b].astype(np.float32) @ W2.T.astype(np.float32)
                        + b2.astype(np.float32), 0.0)
        s = (qp @ kp.T) * np.float32(SCALE)
        smax = float(s.max())
        hot = s.max(axis=0) > THETA
        per_batch.append((qp, kp, smax, hot))

    hot_tiles = max(-(-int(h.sum()) // 128) for _, _, _, h in per_batch)
    hot_tiles = max(hot_tiles, 1)
    # each core of a batch pair gets half the cold rows
    cold_tiles = max(-(-(int((~h).sum()) + 1) // 2 // 128)
                     for _, _, _, h in per_batch)
    cold_tiles += cold_tiles % 2

    ccols = C + 1 + RANK
    batch_maps = []
    for b in range(B):
        qp, kp, smax, hot = per_batch[b]
        nhot, nh = int(hot.sum()), hot_tiles * 128
        kh = np.zeros((nh, D), np.float32)
        kh[:nhot] = k[b][hot]
        if nhot < nh:  # pad: duplicate k row, vhot stays 0 -> contributes 0
            kh[nhot:] = k[b][0]
        vh = np.zeros((nh, C + 1), np.float32)
        vh[:nhot, :C] = v[b][hot]
        vh[:nhot, C] = 1.0
        vh = np.ascontiguousarray(
            vh.reshape(hot_tiles, 128, C + 1).transpose(1, 0, 2))

        ncold = int((~hot).sum())
        kpc = kp[~hot]
        A = np.exp(-smax / 2.0)
        ccfull = np.zeros((ncold, ccols), np.float32)
        ccfull[:, :C] = v[b][~hot]
        ccfull[:, C] = 1.0
        Qm = np.empty((NQ_FULL, RANK), np.float32)
        for i, a in enumerate(ALPHAS):
            j = a[0] + a[1] + a[2]
            cj = (pcoef[j] * SCALE ** j * factorial(j)
                  / (factorial(a[0]) * factorial(a[1]) * factorial(a[2])))
            kcol = cj * (kpc[:, 0] ** a[0] * kpc[:, 1] ** a[1]
                         * kpc[:, 2] ** a[2]) * A
            qcol = (qp[:, 0] ** a[0] * qp[:, 1] ** a[1]
                    * qp[:, 2] ** a[2]) * A
            km_ = np.abs(kcol).max() + 1e-300
            qm_ = np.abs(qcol).max() + 1e-300
            t = 2.0 ** np.round(0.5 * np.log2(qm_ / km_))
            ccfull[:, C + 1 + i] = kcol * t
            Qm[:, i] = qcol / t

        ncp = cold_tiles * 128
        cch = []
        for h in range(2):
            part = ccfull[h::2]
            cc = np.zeros((ncp, ccols), np.float32)
            cc[:part.shape[0]] = part
            cc = np.ascontiguousarray(
                cc.reshape(cold_tiles // 2, 2, 128, ccols)
                .transpose(2, 0, 1, 3))
            cch.append(cc.astype(np.float16))

        batch_maps.append({
            "kT9": _t9(kh),
            "vhot": vh.astype(ml_dtypes.bfloat16),
            "shift": np.full((128, 1), -smax, np.float32),
            "ccold": cch,
            "Qm": Qm,
        })

    in_maps = []
    for core in range(8):
        b, h = core // 2, core % 2
        bm = batch_maps[b]
        qs = q[b, h * NQ:(h + 1) * NQ, :]
        qmono = np.ascontiguousarray(
            bm["Qm"][h * NQ:(h + 1) * NQ, :].T).astype(np.float16)
        in_maps.append({
            "qT9": _t9(qs), "qmono": qmono,
            "wq": wq_l, "wk": wk_l, "bq": bq_r, "bk": bk_r,
            "kT9": bm["kT9"], "vhot": bm["vhot"], "shift": bm["shift"],
            "ccold": bm["ccold"][h],
        })
    return in_maps, hot_tiles, cold_tiles


_NC_CACHE = {}


def kernel(q, k, v, W1, b1, W2, b2, _trace=False):
    q, k, v = np.asarray(q), np.asarray(k), np.asarray(v)
    W1, b1 = np.asarray(W1), np.asarray(b1)
    W2, b2 = np.asarray(W2), np.asarray(b2)

    in_maps, hot_tiles, cold_tiles = _host_prep(q, k, v, W1, b1, W2, b2)
    key = (hot_tiles, cold_tiles)
    if key not in _NC_CACHE:
        _NC_CACHE[key] = build_nc(hot_tiles=hot_tiles, cold_tiles=cold_tiles)
    nc = _NC_CACHE[key]

    res = run_bass_kernel_spmd(nc, in_maps, list(range(8)), trace=_trace)

    out = np.empty((B, NQ_FULL, C), np.float32)
    for core in range(8):
        b, h = core // 2, core % 2
        out[b, h * NQ:(h + 1) * NQ, :] = res.results[core]["out"]
    if _trace:
        return out, res
    return out
